# revision 28
# baseline (speedup 1.0000x reference)
"""BigBird protein model forward pass on 8 TRN2 NeuronCores.

Sharding: sequence-data-parallel (512 tokens/core, block-permuted so cores 0/7
own the global edge blocks), replicated bf16 weights streamed from HBM, split
K / V / q-edge AllGathers per layer (pipelined under the V-projection, qT and
early-attention compute respectively).

Uniform SPMD program: every core runs 8 gathered 512-key attention slots (its
8 sequence blocks) plus a distributed partial-softmax pass for the two global
edge blocks: each core scores the 128 edge queries against its local 512 keys
only, and the unnormalized ctx numerators + denominators are summed with one
small AllReduce. Cores 0/7 select their edge block's result (per-core 0/1
selection matrix) and substitute it for their slot-0 output; all per-core
differences (gather indices, selection data) enter as input data, so the
reference block-sparse softmax is reproduced exactly (duplicate gathered
blocks appear naturally in the gathered key list).
"""

import math
import os

import numpy as np
import ml_dtypes

import concourse.bass as bass
import concourse.bacc as bacc
import concourse.mybir as mybir
import concourse.tile as tile
from concourse.bass_utils import run_bass_kernel_spmd
from concourse.masks import make_identity

NCORES = 8
S = 4096
BS = 64
NB = 64          # sequence blocks
H = 1536
NH = 8
HD = 192
FF = 3072
DIN = 1280
NLAYER = 12
TOUT = 256
SH = 512         # tokens per core
NBC = 8          # blocks per core
NEG = -1e9
EPS = 1e-12

BF = mybir.dt.bfloat16
F32 = mybir.dt.float32
I16 = mybir.dt.int16
F8 = mybir.dt.float8e4
AF = mybir.ActivationFunctionType
ALU = mybir.AluOpType

TRACE = False  # set True (or env BB_TRACE=1) to capture a HW profile
_LAST_RESULT = {}


def head_chunks(h):
    """[(chunk j, partition offset, size)] covering features [192h, 192h+192)."""
    f0 = HD * h
    j0, off = f0 // 128, f0 % 128
    if off == 0:
        return [(j0, 0, 128), (j0 + 1, 0, 64)]
    return [(j0, 64, 64), (j0 + 1, 0, 128)]


def build_program(has_mask, ln_trivial, fp8ffn, bias_trivial):
    nc = bacc.Bacc("TRN2", target_bir_lowering=False, debug=False,
                   num_devices=NCORES)

    def inp(name, shape, dtype=BF):
        return nc.declare_dram_parameter(name, list(shape), dtype, isOutput=False)

    xT = inp("xT", [DIN, SH])
    pos = inp("pos", [SH, H], F32)
    Wproj = inp("Wproj", [DIN + 1, H])
    Wq = inp("Wq", [NLAYER, H, H])
    BQ = inp("BQ", [NLAYER, 128, 12], F32)
    Wkv = inp("Wkv", [NLAYER, H + 1, 2 * H])
    Wo = inp("Wo", [NLAYER, H + 1, H])
    BI2 = inp("BI2", [NLAYER, 128, 24], F32)
    if fp8ffn:
        Wi8 = inp("Wi8", [NLAYER, H, FF], F8)
        Wo28 = inp("Wo28", [NLAYER, FF, H], F8)
        WSCL = inp("WSCL", [NLAYER, 128, 2], F32)
    else:
        Wi = inp("Wi", [NLAYER, H + 1, FF])
        Wo2 = inp("Wo2", [NLAYER, FF + 1, H])
    CW1 = inp("CW1", [H + 1, 512])
    CW2 = inp("CW2", [513, TOUT])
    KIDX = inp("KIDX", [128, 9 * 32], I16)
    VIDX = inp("VIDX", [128, 9 * 32], I16)
    SMAT = inp("SMAT", [128, 64])
    OMS = inp("OMS", [128, 1], F32)
    if has_mask:
        BMID = inp("BMID", [1, 9 * 512])
    if not ln_trivial:
        EMBLN = inp("EMBLN", [2, H], F32)
        LN1 = inp("LN1", [NLAYER, 2, H], F32)
        LN2 = inp("LN2", [NLAYER, 2, H], F32)
    out = nc.declare_dram_parameter("out", [SH, TOUT], F32, isOutput=True)

    with tile.TileContext(nc) as tc:
        const = tc.alloc_tile_pool(name="const", bufs=1)
        wk = tc.alloc_tile_pool(name="wk", bufs=12)
        waug = tc.alloc_tile_pool(name="waug", bufs=2)
        bigact = tc.alloc_tile_pool(name="bigact", bufs=1)
        gat = tc.alloc_tile_pool(name="gat", bufs=1)
        mwork = tc.alloc_tile_pool(name="mwork", bufs=1)
        smp = tc.alloc_tile_pool(name="smp", bufs=1)
        small = tc.alloc_tile_pool(name="small", bufs=1)
        psp = tc.alloc_tile_pool(name="psp", bufs=1, space="PSUM")
        dram = tc.alloc_tile_pool(name="dram", bufs=1, space="DRAM")

        ident = const.tile([128, 128], BF)
        make_identity(nc, ident[:])
        ones_bf = const.tile([1, 512], BF)
        nc.vector.memset(ones_bf[:], 1.0)
        eps_t = const.tile([128, 1], F32)
        nc.vector.memset(eps_t[:], EPS)
        kidx_sb = const.tile([128, 9 * 32], I16)
        nc.sync.dma_start(kidx_sb[:], KIDX[:])
        vidx_sb = const.tile([128, 9 * 32], I16)
        nc.sync.dma_start(vidx_sb[:], VIDX[:])
        smat_sb = const.tile([128, 64], BF)
        nc.sync.dma_start(smat_sb[:], SMAT[:])
        oms_sb = const.tile([128, 1], F32)
        nc.sync.dma_start(oms_sb[:], OMS[:])

        def ln_bcast(src2xH, which):
            """[2,H] f32 -> two [128,H] broadcast tiles (scale,bias)."""
            ts_ = mwork.tile([128, H], F32, name=f"lns_{which}", tag="lnbc", bufs=4)
            tb_ = mwork.tile([128, H], F32, name=f"lnb_{which}", tag="lnbc", bufs=4)
            for t_, r in ((ts_, 0), (tb_, 1)):
                src = src2xH[r:r + 1, :]
                bcast = bass.AP(tensor=src.tensor, offset=src.offset,
                                ap=[[0, 128]] + list(src.ap[1:]))
                nc.sync.dma_start(t_[:], bcast)
            return ts_, tb_

        def emit_ln(x_m, sb_pair):
            """In-place layernorm of x_m [128, H] f32 over the free dim."""
            stats = small.tile([128, 3, 6], F32, name="bnst", tag="bnst", bufs=3)
            xg = x_m[:].rearrange("p (n f) -> p n f", f=512)
            for i in range(3):
                nc.vector.bn_stats(out=stats[:, i, :], in_=xg[:, i, :])
            mv = small.tile([128, 2], F32, name="bnmv", tag="bnmv", bufs=3)
            nc.vector.bn_aggr(out=mv[:], in_=stats[:])
            rstd = small.tile([128, 1], F32, name="rstd", tag="rstd", bufs=3)
            nc.scalar.activation(rstd[:], mv[:, 1:2], AF.Sqrt, bias=eps_t[:])
            nc.vector.reciprocal(rstd[:], rstd[:])
            nc.vector.tensor_scalar(out=x_m[:], in0=x_m[:], scalar1=mv[:, 0:1],
                                    scalar2=rstd[:], op0=ALU.subtract, op1=ALU.mult)
            if sb_pair is not None:
                s_bc, b_bc = sb_pair
                nc.vector.tensor_mul(out=x_m[:], in0=x_m[:], in1=s_bc[:])
                nc.vector.tensor_add(out=x_m[:], in0=x_m[:], in1=b_bc[:])

        def emit_xbt(x_m, dest_xbT, m, scale=None):
            """cast [128,H] f32 -> bf16 (or scaled fp8), transpose into
            dest_xbT[:, :, m*128:+128]."""
            yb = mwork.tile([128, H], BF, name="yb", tag="yb", bufs=1)
            nc.scalar.copy(yb[:], x_m[:])
            for g in range(3):
                tps = psp.tile([128, 4, 128], BF, name="tps", tag="tp", bufs=2)
                for i in range(4):
                    k = 4 * g + i
                    nc.tensor.transpose(tps[:, i, :], yb[:, k * 128:(k + 1) * 128],
                                        ident[:])
                dst = dest_xbT[:, 4 * g:4 * g + 4, m * 128:(m + 1) * 128]
                if scale is None:
                    nc.vector.tensor_copy(dst, tps[:])
                else:
                    nc.vector.tensor_scalar_mul(dst, in0=tps[:], scalar1=scale)

        def load_wk(src2d, rows, cols, name):
            """Load weight k-chunk tiles [128, cols] (+ 1-row aug tile)."""
            nk = rows // 128
            tiles = []
            for k in range(nk):
                t = wk.tile([128, cols], BF, name=f"{name}{k}", tag="wk")
                nc.sync.dma_start(t[:], src2d[k * 128:(k + 1) * 128, :])
                tiles.append(t)
            augt = None
            if rows % 128:
                augt = waug.tile([1, cols], BF, name=f"{name}aug", tag="waug")
                nc.sync.dma_start(augt[:], src2d[nk * 128:nk * 128 + 1, :])
            return tiles, augt

        def store_ctxT_add(ctxn, ctxT, h, sc):
            """ctxn [64,192] bf16 -> transposed, ADDED into ctxT cols sc."""
            f0 = HD * h
            j0, off = f0 // 128, f0 % 128
            tct = psp.tile([128, 2, 64], BF, name="tct", tag="tp", bufs=2)
            nc.tensor.transpose(tct[:, 0, :], ctxn[:, 0:128], ident[0:64, 0:64])
            nc.tensor.transpose(tct[0:64, 1, :], ctxn[:, 128:192],
                                ident[0:64, 0:64])

            def acc(dst, src):
                nc.vector.tensor_tensor(out=dst, in0=dst, in1=src, op=ALU.add)

            if off == 0:
                acc(ctxT[:, j0, sc], tct[:, 0, :])
                acc(ctxT[0:64, j0 + 1, sc], tct[0:64, 1, :])
            else:
                acc(ctxT[64:128, j0, sc], tct[0:64, 0, :])
                acc(ctxT[0:64, j0 + 1, sc], tct[64:128, 0, :])
                acc(ctxT[64:128, j0 + 1, sc], tct[0:64, 1, :])

        # ---------------- embedding ----------------
        scopes = []

        def scope(name):
            scopes.append(name)
            return nc.named_scope(name)

        xt_sb = bigact.tile([128, 10, SH], BF, name="xt0", tag="xbt", bufs=2)
        nc.sync.dma_start(xt_sb[:], xT.rearrange("(k p) t -> p k t", p=128)[:])
        pw, pwa = load_wk(Wproj[:], DIN + 1, H, "wp")
        emb_bc = None if ln_trivial else ln_bcast(EMBLN[:], "emb")

        resid = dram.tile([SH, H], F32, name="resid0", tag="resid", bufs=2)
        xbT = bigact.tile([128, 12, SH], BF, name="xbT0", tag="xbt", bufs=2)
        for m in range(4):
            ms = slice(m * 128, (m + 1) * 128)
            x_m = mwork.tile([128, H], F32, name="xemb", tag="x2", bufs=1)
            for n in range(3):
                ns = slice(n * 512, (n + 1) * 512)
                pm = psp.tile([128, 512], F32, name="pmm", tag="mm", bufs=2)
                for k in range(10):
                    nc.tensor.matmul(pm[:], lhsT=xt_sb[:, k, ms], rhs=pw[k][:, ns],
                                     start=(k == 0),
                                     stop=(bias_trivial and k == 9))
                if not bias_trivial:
                    nc.tensor.matmul(pm[:], lhsT=ones_bf[0:1, 0:128],
                                     rhs=pwa[0:1, ns], start=False, stop=True)
                posr = mwork.tile([128, 512], F32, name="posr", tag="xmn", bufs=2)
                nc.sync.dma_start(posr[:], pos[ms, ns])
                nc.vector.tensor_add(out=x_m[:, ns], in0=pm[:], in1=posr[:])
            emit_ln(x_m, emb_bc)
            nc.sync.dma_start(resid[ms, :], x_m[:])
            emit_xbt(x_m, xbT, m)

        # ---------------- layers ----------------
        for layer in range(NLAYER):
            # ---- Phase A: K then V projections. K AllGathers first (its
            # gathered keys feed the first score matmuls); the V AllGather
            # overlaps qT; the small q-edge broadcast overlaps early slots. ----
            kin = dram.tile([SH, H], BF, name=f"kin{layer}", tag="kin", bufs=2)
            vin = dram.tile([SH, H], BF, name=f"vin{layer}", tag="vin", bufs=2)
            kfull = dram.tile([S, H], BF, name=f"kfull{layer}", tag="kfull",
                              bufs=2, addr_space="Shared")
            vfull = dram.tile([S, H], BF, name=f"vfull{layer}", tag="vfull",
                              bufs=2, addr_space="Shared")
            with scope(f"L{layer}_kv"):
                for half, dst, full in ((0, kin, kfull), (1, vin, vfull)):
                    hs = slice(half * H, (half + 1) * H)
                    wt, wta = load_wk(Wkv[layer, :, hs], H + 1, H, f"wkv{half}")
                    for m in range(4):
                        ms = slice(m * 128, (m + 1) * 128)
                        kv_sb = mwork.tile([128, 3, 512], BF, name="kvsb", tag="kvsb",
                                           bufs=1)
                        for n in range(3):
                            ns = slice(n * 512, (n + 1) * 512)
                            pm = psp.tile([128, 512], F32, name="pmkv", tag="mm",
                                          bufs=2)
                            for k in range(12):
                                nc.tensor.matmul(pm[:], lhsT=xbT[:, k, ms],
                                                 rhs=wt[k][:, ns],
                                                 start=(k == 0),
                                                 stop=(bias_trivial and k == 11))
                            if not bias_trivial:
                                nc.tensor.matmul(pm[:], lhsT=ones_bf[0:1, 0:128],
                                                 rhs=wta[0:1, ns], start=False,
                                                 stop=True)
                            nc.scalar.copy(kv_sb[:, n, :], pm[:])
                        nc.sync.dma_start(dst[ms, :], kv_sb[:])
                    nc.gpsimd.collective_compute(
                        "AllGather", ALU.bypass, ins=[dst.opt()],
                        outs=[full.opt()],
                        replica_groups=[list(range(NCORES))])

            # ---- Phase A3: qT (overlaps the V AllGather) ----
            with scope(f"L{layer}_q"):
                wtq, _ = load_wk(Wq[layer], H, H, "wq")
                bq_sb = small.tile([128, 12], F32, name="bq", tag="bq", bufs=2)
                nc.sync.dma_start(bq_sb[:], BQ[layer])
                qT = bigact.tile([128, 12, SH], BF, name=f"qT{layer}", tag="qt",
                                 bufs=1)
                for j in range(12):
                    js = slice(j * 128, (j + 1) * 128)
                    pm = psp.tile([128, 512], F32, name="pmq", tag="mm", bufs=2)
                    for k in range(12):
                        nc.tensor.matmul(pm[:], lhsT=wtq[k][:, js], rhs=xbT[:, k, :],
                                         start=(k == 0), stop=(k == 11))
                    nc.scalar.activation(qT[:, j, :], pm[:], AF.Identity,
                                         bias=bq_sb[:, j:j + 1])
                # broadcast the two edge blocks' q (slot-0 cols of cores 0/7)
                qein = dram.tile([H, 64], BF, name=f"qein{layer}", tag="qein",
                                 bufs=2)
                qeall = dram.tile([NCORES * H, 64], BF, name=f"qeall{layer}",
                                  tag="qeall", bufs=2, addr_space="Shared")
                nc.sync.dma_start(
                    qein[:].rearrange("(j p) q -> p j q", p=128), qT[:, :, 0:64])
                nc.gpsimd.collective_compute(
                    "AllGather", ALU.bypass, ins=[qein.opt()], outs=[qeall.opt()],
                    replica_groups=[list(range(NCORES))])

            # ---- Phase B: attention ----
            ctxT = bigact.tile([128, 12, SH], BF, name=f"ctxT{layer}", tag="ctxt",
                               bufs=1)
            with scope(f"L{layer}_attn"):
                # Softmax without max-subtraction: scores for this model are
                # O(1) (LN'd activations, 0.02-std weights), so exp() cannot
                # overflow and -1e9 mask biases underflow to exactly 0. The
                # edge blocks' softmax distributes across cores as plain sums.
                # edge partial payload: per head h, cols [h*193, h*193+193) =
                # [192 ctx numerator | 1 row-sum], rows = eb*64 + query
                eall = dram.tile([128, NH * (HD + 1)], F32, name=f"eall{layer}",
                                 tag="eall", bufs=2)
                easall = dram.tile([128, NH * (HD + 1)], F32,
                                   name=f"easall{layer}", tag="easall", bufs=2,
                                   addr_space="Shared")
                def emit_scores(gid, eb, sc, h2, qsrc, ktg, bch):
                    """Scores + softmax for one (gid, eb, h2); returns p_sb."""
                    jA, jM, jB = 3 * h2, 3 * h2 + 1, 3 * h2 + 2
                    sps = psp.tile([128, 512], F32, name="sps", tag="s",
                                   bufs=2)
                    nc.tensor.matmul(
                        sps[0:64, :], lhsT=qsrc[:, jA, sc],
                        rhs=ktg[:, jA, :], tile_position=(0, 0),
                        start=True, stop=False)
                    nc.tensor.matmul(
                        sps[0:64, :], lhsT=qsrc[0:64, jM, sc],
                        rhs=ktg[0:64, jM, :], tile_position=(0, 0),
                        start=False, stop=not has_mask)
                    nc.tensor.matmul(
                        sps[64:128, :], lhsT=qsrc[64:128, jM, sc],
                        rhs=ktg[64:128, jM, :], tile_position=(64, 64),
                        start=True, stop=False)
                    nc.tensor.matmul(
                        sps[64:128, :], lhsT=qsrc[:, jB, sc],
                        rhs=ktg[:, jB, :], tile_position=(0, 64),
                        start=False, stop=not has_mask)
                    if has_mask:
                        nc.tensor.matmul(
                            sps[0:64, :], lhsT=ones_bf[0:1, 0:64],
                            rhs=bch[0:1, :], tile_position=(0, 0),
                            start=False, stop=True)
                        nc.tensor.matmul(
                            sps[64:128, :], lhsT=ones_bf[0:1, 0:64],
                            rhs=bch[0:1, :], tile_position=(0, 64),
                            start=False, stop=True)
                    p_sb = smp.tile([128, 512], BF, name="psb", tag="psb",
                                    bufs=2)
                    rs = small.tile([128, 1], F32, name="rs", tag="rs",
                                    bufs=3)
                    nc.scalar.activation(p_sb[:], sps[:], AF.Exp,
                                         accum_out=rs[:])
                    if eb is None:
                        # normalize p up front: ctx comes out final
                        rcp = small.tile([128, 1], F32, name="rcp",
                                         tag="rcp", bufs=3)
                        nc.vector.reciprocal(rcp[:], rs[:])
                        nc.vector.tensor_scalar_mul(p_sb[:], in0=p_sb[:],
                                                    scalar1=rcp[:])
                    else:
                        # edge: keep p unnormalized; ship row sums
                        # straight into the AllReduce payload
                        es = slice(eb * 64, (eb + 1) * 64)
                        cA = 2 * h2 * (HD + 1) + HD
                        cB = (2 * h2 + 1) * (HD + 1) + HD
                        nc.sync.dma_start(eall[es, cA:cA + 1], rs[0:64, :])
                        nc.sync.dma_start(eall[es, cB:cB + 1], rs[64:128, :])
                    return p_sb

                def emit_ctx(gid, eb, sc, h2, p_sb, vg):
                    """Transpose p + ctx matmuls for one queued (gid, eb, h2)."""
                    ptps = psp.tile([128, 4, 128], BF, name="ptps",
                                    tag="tp", bufs=2)
                    for kc in range(4):
                        nc.tensor.transpose(
                            ptps[:, kc, :],
                            p_sb[:, kc * 128:(kc + 1) * 128], ident[:])
                    pt_sb = smp.tile([128, 4, 128], BF, name="ptsb",
                                     tag="ptsb", bufs=2)
                    nc.vector.tensor_copy(pt_sb[:], ptps[:])
                    if eb is None:
                        # p is pre-normalized: compute ctx directly in
                        # ctxT's [feature, query] layout; the pair spans
                        # ctxT chunks 3*h2..3*h2+2 exactly.
                        fA = 2 * h2 * HD
                        fB = fA + HD
                        ctxps = psp.tile([128, 3, 64], F32, name="ctxps",
                                         tag="ctx", bufs=2)
                        for kc in range(4):
                            nc.tensor.matmul(
                                ctxps[:, 0, :],
                                lhsT=vg[:, kc, fA:fA + 128],
                                rhs=pt_sb[:, kc, 0:64],
                                start=(kc == 0), stop=(kc == 3))
                        for kc in range(4):
                            nc.tensor.matmul(
                                ctxps[0:64, 1, :],
                                lhsT=vg[:, kc, fA + 128:fA + 192],
                                rhs=pt_sb[:, kc, 0:64],
                                tile_position=(0, 0),
                                start=(kc == 0), stop=(kc == 3))
                        for kc in range(4):
                            nc.tensor.matmul(
                                ctxps[64:128, 1, :],
                                lhsT=vg[:, kc, fB:fB + 64],
                                rhs=pt_sb[:, kc, 64:128],
                                tile_position=(0, 64),
                                start=(kc == 0), stop=(kc == 3))
                        for kc in range(4):
                            nc.tensor.matmul(
                                ctxps[:, 2, :],
                                lhsT=vg[:, kc, fB + 64:fB + 192],
                                rhs=pt_sb[:, kc, 64:128],
                                start=(kc == 0), stop=(kc == 3))
                        if gid == 0:
                            # cores 0/7: slot-0 gathered result is a
                            # placeholder -> zero it (OMS=0); edge ctx
                            # is added afterwards.
                            nc.vector.tensor_scalar_mul(
                                ctxT[:, 3 * h2:3 * h2 + 3, sc],
                                in0=ctxps[:], scalar1=oms_sb[:])
                        else:
                            nc.vector.tensor_copy(
                                ctxT[:, 3 * h2:3 * h2 + 3, sc], ctxps[:])
                    else:
                        # edge partial ctx (query-major, unnormalized)
                        es = slice(eb * 64, (eb + 1) * 64)
                        for hh in range(2):
                            h = 2 * h2 + hh
                            qs = slice(hh * 64, (hh + 1) * 64)
                            cps = psp.tile([64, HD], F32, name="cps",
                                           tag="ctx", bufs=2)
                            for kc in range(4):
                                nc.tensor.matmul(
                                    cps[:], lhsT=pt_sb[:, kc, qs],
                                    rhs=vg[:, kc, h * HD:(h + 1) * HD],
                                    start=(kc == 0), stop=(kc == 3))
                            stg = smp.tile([64, HD], F32, name="estg",
                                           tag="estg", bufs=2)
                            nc.vector.tensor_copy(stg[:], cps[:])
                            c0 = h * (HD + 1)
                            nc.sync.dma_start(eall[es, c0:c0 + HD], stg[:])

                # software pipeline: scores of work item i+1 are emitted
                # before transposes+ctx of item i, so the PE queue never
                # head-of-line blocks on the softmax (Scalar/Vector) latency
                pending = None
                for gid in (0, 1, 2, 3, 4, 5, 6, 7, 8):
                    gsl = slice(gid * 32, (gid + 1) * 32)
                    ktg = gat.tile([128, 12, 512], BF, name="ktg", tag="ktg",
                                   bufs=2)
                    nc.gpsimd.dma_gather(
                        out_ap=ktg[:], in_ap=kfull[:] if gid < 8 else kin[:],
                        idxs_ap=kidx_sb[:, gsl], num_idxs=512, num_idxs_reg=512,
                        elem_size=H, elem_step=H, transpose=True)
                    vg = gat.tile([128, 4, H], BF, name="vg", tag="vg", bufs=2)
                    nc.gpsimd.dma_gather(
                        out_ap=vg[:], in_ap=vfull[:] if gid < 8 else vin[:],
                        idxs_ap=vidx_sb[:, gsl], num_idxs=512, num_idxs_reg=512,
                        elem_size=H, elem_step=H)
                    bch = None
                    if has_mask:
                        bch = small.tile([1, 512], BF, name="bch", tag="bch",
                                         bufs=2)
                        nc.sync.dma_start(
                            bch[:], BMID[0:1, gid * 512:(gid + 1) * 512])
                    if gid < 8:
                        sc = slice(gid * 64, (gid + 1) * 64)
                        eb_list = [(None, sc)]
                        qsrc = qT
                    else:
                        # receive the two edge q blocks (feature-major)
                        qe_sb = smp.tile([128, 12, 128], BF, name="qe", tag="qe",
                                         bufs=1)
                        nc.sync.dma_start(
                            qe_sb[:, :, 0:64],
                            qeall[0:H, :].rearrange("(j p) q -> p j q", p=128))
                        nc.sync.dma_start(
                            qe_sb[:, :, 64:128],
                            qeall[7 * H:8 * H, :].rearrange("(j p) q -> p j q",
                                                            p=128))
                        eb_list = [(0, slice(0, 64)), (1, slice(64, 128))]
                        qsrc = qe_sb
                    for eb, sc in eb_list:
                        for h2 in range(NH // 2):
                            p_sb = emit_scores(gid, eb, sc, h2, qsrc, ktg, bch)
                            if pending is not None:
                                emit_ctx(*pending)
                            pending = (gid, eb, sc, h2, p_sb, vg)
                if pending is not None:
                    emit_ctx(*pending)
                # ---- AllReduce edge numerators + denominators, then merge ----
                nc.gpsimd.collective_compute(
                    "AllReduce", ALU.add, ins=[eall.opt()], outs=[easall.opt()],
                    replica_groups=[list(range(NCORES))])
                for h in range(NH):
                    eh = mwork.tile([128, HD + 1], F32, name="easum",
                                    tag="easum", bufs=2)
                    nc.sync.dma_start(eh[:], easall[:, h * (HD + 1):
                                                    (h + 1) * (HD + 1)])
                    ctxn = smp.tile([128, HD], BF, name="ctxn0", tag="ctxn",
                                    bufs=2)
                    for eb in range(2):
                        es = slice(eb * 64, (eb + 1) * 64)
                        rcp = small.tile([128, 1], F32, name="rcp0", tag="rcp",
                                         bufs=3)
                        nc.vector.reciprocal(rcp[es, :], eh[es, HD:HD + 1])
                        nc.vector.tensor_scalar_mul(
                            ctxn[es, :], in0=eh[es, 0:HD],
                            scalar1=rcp[es, :])
                    # per-core row selection: own edge block's 64 queries (or
                    # all-zero on cores 1-6)
                    ops2 = psp.tile([64, HD], F32, name="ops2", tag="ctx",
                                    bufs=2)
                    nc.tensor.matmul(ops2[:], lhsT=smat_sb[:], rhs=ctxn[:],
                                     start=True, stop=True)
                    own = smp.tile([64, HD], BF, name="own", tag="ctxn", bufs=2)
                    nc.scalar.copy(own[:], ops2[:])
                    store_ctxT_add(own, ctxT, h, slice(0, 64))

            # ---- Phase C: Wo + residual + LN1 ----
            with scope(f"L{layer}_wo"):
                wto, wtoa = load_wk(Wo[layer], H + 1, H, "wo")
                ln1_bc = None if ln_trivial else ln_bcast(LN1[layer], f"l1_{layer}")
                x2d = dram.tile([SH, H], F32, name=f"x2d{layer}", tag="x2d", bufs=2)
                x2bT = bigact.tile([128, 12, SH], F8 if fp8ffn else BF,
                                   name=f"x2bT{layer}", tag="xbt", bufs=2)
                # m=0 last: its ctxT cols include slot 0, which waits on the
                # edge AllReduce merge — the other chunks hide that latency.
                for m in (1, 2, 3, 0):
                    ms = slice(m * 128, (m + 1) * 128)
                    x2_m = mwork.tile([128, H], F32, name="x2m", tag="x2", bufs=1)
                    for n in range(3):
                        ns = slice(n * 512, (n + 1) * 512)
                        pm = psp.tile([128, 512], F32, name="pmo", tag="mm", bufs=2)
                        for k in range(12):
                            nc.tensor.matmul(pm[:], lhsT=ctxT[:, k, ms],
                                             rhs=wto[k][:, ns],
                                             start=(k == 0),
                                             stop=(bias_trivial and k == 11))
                        if not bias_trivial:
                            nc.tensor.matmul(pm[:], lhsT=ones_bf[0:1, 0:128],
                                             rhs=wtoa[0:1, ns], start=False,
                                             stop=True)
                        xr = mwork.tile([128, 512], F32, name="xr", tag="xmn", bufs=2)
                        nc.sync.dma_start(xr[:], resid[ms, ns])
                        nc.vector.tensor_add(out=x2_m[:, ns], in0=pm[:], in1=xr[:])
                    emit_ln(x2_m, ln1_bc)
                    nc.sync.dma_start(x2d[ms, :], x2_m[:])
                    emit_xbt(x2_m, x2bT, m, scale=32.0 if fp8ffn else None)

            # ---- Phase D: FFN (h1T computed feature-major, like qT) ----
            with scope(f"L{layer}_ffn"):
                h1sb = bigact.tile([128, 24, SH], BF, name=f"h1sb{layer}",
                                   tag="h1sb", bufs=1)
                bi_sb = small.tile([128, 24], F32, name="bi2", tag="bi2", bufs=2)
                nc.sync.dma_start(bi_sb[:], BI2[layer])
                if fp8ffn:
                    wscl_sb = small.tile([128, 2], F32, name="wscl", tag="wscl",
                                         bufs=2)
                    nc.sync.dma_start(wscl_sb[:], WSCL[layer])
                for half in range(2):
                    hs = slice(half * H, (half + 1) * H)
                    if fp8ffn:
                        wti2 = []
                        for k2 in range(6):
                            t = wk.tile([128, 2, H], F8, name=f"wi8_{half}_{k2}",
                                        tag="wk")
                            nc.sync.dma_start(
                                t[:], Wi8[layer, k2 * 256:(k2 + 1) * 256, hs]
                                .rearrange("(i p) c -> p i c", p=128)[:])
                            wti2.append(t)
                    else:
                        wti, _ = load_wk(Wi[layer, 0:H, hs], H, H, f"wi{half}")
                    for j in range(12):
                        js = slice(j * 128, (j + 1) * 128)
                        pm = psp.tile([128, 512], F32, name="pmi", tag="mm",
                                      bufs=2)
                        if fp8ffn:
                            for k2 in range(6):
                                nc.tensor.matmul(
                                    pm[:], lhsT=wti2[k2][:, :, js],
                                    rhs=x2bT[:, 2 * k2:2 * k2 + 2, :],
                                    perf_mode=mybir.MatmulPerfMode.DoubleRow,
                                    start=(k2 == 0), stop=(k2 == 5))
                        else:
                            for k in range(12):
                                nc.tensor.matmul(pm[:], lhsT=wti[k][:, js],
                                                 rhs=x2bT[:, k, :],
                                                 start=(k == 0), stop=(k == 11))
                        jg = half * 12 + j
                        nc.scalar.activation(h1sb[:, jg, :], pm[:],
                                             AF.Gelu_apprx_tanh,
                                             bias=bi_sb[:, jg:jg + 1])
                ln2_bc = None if ln_trivial else ln_bcast(LN2[layer], f"l2_{layer}")
                x3d = dram.tile([SH, H], F32, name=f"x3d{layer}", tag="x3d", bufs=2)
                for nr in range(3):
                    ns = slice(nr * 512, (nr + 1) * 512)
                    wt2 = []
                    for k2 in range(12):
                        t = wk.tile([128, 2, 512], BF, name=f"wo2_{k2}", tag="wk")
                        nc.sync.dma_start(
                            t[:], Wo2[layer, k2 * 256:(k2 + 1) * 256, ns]
                            .rearrange("(i p) c -> p i c", p=128)[:])
                        wt2.append(t)
                    wt2a = waug.tile([1, 512], BF, name="wo2aug", tag="waug")
                    nc.sync.dma_start(wt2a[:], Wo2[layer, FF:FF + 1, ns])
                    for m in range(4):
                        ms = slice(m * 128, (m + 1) * 128)
                        pm = psp.tile([128, 512], F32, name="pm2", tag="mm", bufs=2)
                        for k in range(24):
                            nc.tensor.matmul(pm[:], lhsT=h1sb[:, k, ms],
                                             rhs=wt2[k // 2][:, k % 2, :],
                                             start=(k == 0),
                                             stop=(bias_trivial and k == 23))
                        if not bias_trivial:
                            nc.tensor.matmul(pm[:], lhsT=ones_bf[0:1, 0:128],
                                             rhs=wt2a[0:1, :], start=False,
                                             stop=True)
                        xmn = mwork.tile([128, 512], F32, name="xmn", tag="xmn",
                                         bufs=2)
                        nc.sync.dma_start(xmn[:], x2d[ms, ns])
                        x3s = mwork.tile([128, 512], F32, name="x3s", tag="x3s",
                                         bufs=2)
                        nc.vector.tensor_add(out=x3s[:], in0=pm[:],
                                             in1=xmn[:])
                        nc.sync.dma_start(x3d[ms, ns], x3s[:])
                resid_n = dram.tile([SH, H], F32, name=f"resid{layer + 1}",
                                    tag="resid", bufs=2)
                xbT_n = bigact.tile([128, 12, SH], BF, name=f"xbT{layer + 1}",
                                    tag="xbt", bufs=2)
                for m in range(4):
                    ms = slice(m * 128, (m + 1) * 128)
                    x3m = mwork.tile([128, H], F32, name="x3m", tag="x2", bufs=1)
                    nc.sync.dma_start(x3m[:], x3d[ms, :])
                    emit_ln(x3m, ln2_bc)
                    nc.sync.dma_start(resid_n[ms, :], x3m[:])
                    emit_xbt(x3m, xbT_n, m)
            resid = resid_n
            xbT = xbT_n

        # ---------------- classifier ----------------
        c1t = []
        for k2 in range(6):
            t = wk.tile([128, 2, 512], BF, name=f"cw1_{k2}", tag="wk")
            nc.sync.dma_start(
                t[:], CW1[k2 * 256:(k2 + 1) * 256, :]
                .rearrange("(i p) c -> p i c", p=128)[:])
            c1t.append(t)
        c1a = waug.tile([1, 512], BF, name="cw1aug", tag="waug")
        nc.sync.dma_start(c1a[:], CW1[H:H + 1, :])
        c2t = []
        for k2 in range(2):
            t = wk.tile([128, 2, TOUT], BF, name=f"cw2_{k2}", tag="wk")
            nc.sync.dma_start(
                t[:], CW2[k2 * 256:(k2 + 1) * 256, :]
                .rearrange("(i p) c -> p i c", p=128)[:])
            c2t.append(t)
        c2a = waug.tile([1, TOUT], BF, name="cw2aug", tag="waug")
        nc.sync.dma_start(c2a[:], CW2[512:513, :])
        for m in range(4):
            ms = slice(m * 128, (m + 1) * 128)
            pm = psp.tile([128, 512], F32, name="pmc1", tag="mm", bufs=2)
            for k in range(12):
                nc.tensor.matmul(pm[:], lhsT=xbT[:, k, ms],
                                 rhs=c1t[k // 2][:, k % 2, :],
                                 start=(k == 0), stop=(bias_trivial and k == 11))
            if not bias_trivial:
                nc.tensor.matmul(pm[:], lhsT=ones_bf[0:1, 0:128],
                                 rhs=c1a[0:1, :], start=False, stop=True)
            hb = mwork.tile([128, 512], BF, name="hb", tag="gb", bufs=2)
            nc.scalar.activation(hb[:], pm[:], AF.Relu)
            tps = psp.tile([128, 4, 128], BF, name="tpsc", tag="tp", bufs=2)
            for i in range(4):
                nc.tensor.transpose(tps[:, i, :], hb[:, i * 128:(i + 1) * 128],
                                    ident[:])
            hT = mwork.tile([128, 4, 128], BF, name="hT", tag="th", bufs=2)
            nc.vector.tensor_copy(hT[:], tps[:])
            pm2 = psp.tile([128, TOUT], F32, name="pmc2", tag="mm", bufs=2)
            for k in range(4):
                nc.tensor.matmul(pm2[:], lhsT=hT[:, k, :],
                                 rhs=c2t[k // 2][:, k % 2, :],
                                 start=(k == 0), stop=(bias_trivial and k == 3))
            if not bias_trivial:
                nc.tensor.matmul(pm2[:], lhsT=ones_bf[0:1, 0:128],
                                 rhs=c2a[0:1, :], start=False, stop=True)
            ob = mwork.tile([128, TOUT], F32, name="ob", tag="ob", bufs=2)
            nc.vector.tensor_copy(ob[:], pm2[:])
            nc.sync.dma_start(out[ms, :], ob[:])

        for p in (dram, psp, small, smp, mwork, gat, bigact, waug, wk, const):
            p.release()

    nc.compile()
    return nc


def _core_blocks():
    """blocks[c] = ordered block list for core c; slot 0 is the full-attn slot."""
    blocks = []
    for c in range(NCORES):
        if c == 0:
            blocks.append(list(range(0, 8)))
        elif c == NCORES - 1:
            blocks.append([63] + list(range(56, 63)))
        else:
            blocks.append(list(range(8 * c, 8 * c + 8)))
    return blocks


def _wrap_idx(idx512):
    """[512] int -> [128, 32] wrapped in 16 partitions, replicated x8."""
    w = np.zeros((16, 32), np.int16)
    for i in range(512):
        w[i % 16, i // 16] = idx512[i]
    return np.tile(w, (8, 1))


def kernel(**inputs):
    x = np.asarray(inputs["x"])            # [1, S, DIN] f32
    mask = np.asarray(inputs["attention_mask"]).reshape(-1).astype(np.float64)
    key_blocks = np.asarray(inputs["key_blocks"])  # [62, 8] int32
    scale = 1.0 / math.sqrt(HD)

    blocks = _core_blocks()
    tok_perm = np.concatenate(
        [np.arange(b * BS, (b + 1) * BS) for c in range(NCORES)
         for b in blocks[c]])          # new row -> original token
    blk_pos = np.empty(NB, np.int64)   # block -> position in permuted block order
    for c in range(NCORES):
        for s_, b in enumerate(blocks[c]):
            blk_pos[b] = c * NBC + s_

    has_mask = not bool((mask == 1.0).all())
    ln_trivial = (np.all(np.asarray(inputs["emb_ln_s"]) == 1)
                  and np.all(np.asarray(inputs["emb_ln_b"]) == 0)
                  and np.all(np.asarray(inputs["ln1_s"]) == 1)
                  and np.all(np.asarray(inputs["ln1_b"]) == 0)
                  and np.all(np.asarray(inputs["ln2_s"]) == 1)
                  and np.all(np.asarray(inputs["ln2_b"]) == 0))

    bf = ml_dtypes.bfloat16

    def aug(w, b):
        return np.concatenate([np.asarray(w), np.asarray(b).reshape(1, -1)],
                              axis=0).astype(bf)

    Wq_s = (np.asarray(inputs["Wq"]) * scale).astype(bf)           # [12, H, H]
    BQ = np.stack([np.asarray(inputs["bq"])[ll].reshape(12, 128).T * scale
                   for ll in range(NLAYER)]).astype(np.float32)    # [12,128,12]
    BI2 = np.stack([np.asarray(inputs["bi"])[ll].reshape(24, 128).T
                    for ll in range(NLAYER)]).astype(np.float32)   # [12,128,24]
    fp8ffn = False  # fp8 FFN measured 9.5% rel err (gate 2e-2) - keep bf16
    bias_trivial = all(
        np.all(np.asarray(inputs[k]) == 0)
        for k in ("proj_b", "bk", "bv", "bo", "bo2", "cb1", "cb2"))
    if fp8ffn:
        f8 = ml_dtypes.float8_e4m3
        Wi_f = np.asarray(inputs["Wi"], np.float32)
        Wo2_f = np.asarray(inputs["Wo2"], np.float32)
        swi = 240.0 / np.abs(Wi_f).reshape(NLAYER, -1).max(axis=1)
        sw2 = 240.0 / np.abs(Wo2_f).reshape(NLAYER, -1).max(axis=1)
        Wi8 = (Wi_f * swi[:, None, None]).astype(f8)
        Wo28 = (Wo2_f * sw2[:, None, None]).astype(f8)
        WSCL = np.zeros((NLAYER, 128, 2), np.float32)
        WSCL[:, :, 0] = (1.0 / (swi * 32.0))[:, None]
        WSCL[:, :, 1] = (1.0 / sw2)[:, None]
    Wkv = np.concatenate(
        [np.concatenate([np.asarray(inputs["Wk"]),
                         np.asarray(inputs["Wv"])], axis=2),
         np.concatenate([np.asarray(inputs["bk"])[:, None, :],
                         np.asarray(inputs["bv"])[:, None, :]], axis=2)],
        axis=1).astype(bf)                                         # [12, H+1, 2H]
    Wo_a = np.concatenate([np.asarray(inputs["Wo"]),
                           np.asarray(inputs["bo"])[:, None, :]],
                          axis=1).astype(bf)
    Wi_a = np.concatenate([np.asarray(inputs["Wi"]),
                           np.asarray(inputs["bi"])[:, None, :]],
                          axis=1).astype(bf)
    Wo2_a = np.concatenate([np.asarray(inputs["Wo2"]),
                            np.asarray(inputs["bo2"])[:, None, :]],
                           axis=1).astype(bf)
    Wproj_a = aug(inputs["proj_w"], inputs["proj_b"])
    CW1 = aug(inputs["cw1"], inputs["cb1"])
    CW2 = aug(inputs["cw2"], inputs["cb2"])
    pos_full = (np.asarray(inputs["pos_emb"]) +
                np.asarray(inputs["tok_emb"])[None, :]).astype(np.float32)

    mask_bias_tok = NEG * (1.0 - mask)     # per original token

    in_maps = []
    for c in range(NCORES):
        toks = tok_perm[c * SH:(c + 1) * SH]
        xT_c = np.ascontiguousarray(x[0, toks, :].astype(bf).T)    # [DIN, 512]
        pos_c = np.ascontiguousarray(pos_full[toks])               # [512, H]

        own_idx = np.arange(c * SH, (c + 1) * SH)
        # gather indices: ids 0..7 = per-slot gathered key rows (slot 0 of
        # cores 0/7 is a discarded placeholder -> own rows), id 8 = local rows
        # (edge-partial K/V read from kin/vin)
        kidx = np.zeros((128, 9 * 32), np.int16)
        vidx = np.zeros((128, 9 * 32), np.int16)
        slot_rows = []
        for s_ in range(8):
            b = blocks[c][s_]
            if 1 <= b <= 62:
                row = key_blocks[b - 1]                            # 8 block ids
                idx = np.concatenate(
                    [np.arange(blk_pos[kb] * BS, (blk_pos[kb] + 1) * BS)
                     for kb in row])
            else:                          # global edge block: placeholder
                idx = own_idx
            slot_rows.append(idx)
            kidx[:, s_ * 32:(s_ + 1) * 32] = _wrap_idx(idx)
            vidx[:, s_ * 32:(s_ + 1) * 32] = _wrap_idx(idx)
        kidx[:, 8 * 32:9 * 32] = _wrap_idx(np.arange(SH))
        vidx[:, 8 * 32:9 * 32] = _wrap_idx(np.arange(SH))

        # SMAT: row-selection for this core's edge block (eb0 rows 0-63 for
        # core 0, eb1 rows 64-127 for core 7, zero elsewhere)
        smat = np.zeros((128, 64), np.float32)
        if c == 0:
            smat[np.arange(64), np.arange(64)] = 1.0
        elif c == NCORES - 1:
            smat[64 + np.arange(64), np.arange(64)] = 1.0
        oms = np.full((128, 1), 0.0 if c in (0, NCORES - 1) else 1.0,
                      np.float32)

        im = dict(xT=xT_c, pos=pos_c, Wproj=Wproj_a, Wq=Wq_s, BQ=BQ, Wkv=Wkv,
                  Wo=Wo_a, BI2=BI2, CW1=CW1, CW2=CW2,
                  KIDX=kidx, VIDX=vidx, SMAT=smat.astype(bf), OMS=oms)
        if has_mask:
            # BMID: mask bias over gathered keys (slots 0..7) + local keys
            # (edge partial, id 8), in permuted row space
            bm = np.zeros((9, 512), np.float64)
            perm_mask_bias = mask_bias_tok[tok_perm]
            for s_ in range(8):
                bm[s_] = perm_mask_bias[slot_rows[s_]]
            bm[8] = perm_mask_bias[own_idx]
            im["BMID"] = bm.astype(bf).reshape(1, 9 * 512)
        if fp8ffn:
            im.update(Wi8=Wi8, Wo28=Wo28, WSCL=WSCL)
        else:
            im.update(Wi=Wi_a, Wo2=Wo2_a)
        if not ln_trivial:
            im["EMBLN"] = np.stack(
                [np.asarray(inputs["emb_ln_s"]),
                 np.asarray(inputs["emb_ln_b"])]).astype(np.float32)
            im["LN1"] = np.stack([np.asarray(inputs["ln1_s"]),
                                  np.asarray(inputs["ln1_b"])],
                                 axis=1).astype(np.float32)
            im["LN2"] = np.stack([np.asarray(inputs["ln2_s"]),
                                  np.asarray(inputs["ln2_b"])],
                                 axis=1).astype(np.float32)
        in_maps.append(im)

    nc = build_program(has_mask, ln_trivial, fp8ffn, bias_trivial)
    trace = TRACE or bool(int(os.environ.get("BB_TRACE", "0")))
    res = run_bass_kernel_spmd(nc, in_maps, list(range(NCORES)), trace=trace)
    _LAST_RESULT["exec_time_ns"] = res.exec_time_ns
    _LAST_RESULT["profile_json"] = getattr(res, "profile_json", None)
    _LAST_RESULT["scope_times"] = getattr(res, "per_core_scope_times", None)
    _LAST_RESULT["trace"] = getattr(res, "instructions_and_trace", None)

    out = np.empty((S, TOUT), np.float32)
    shards = np.concatenate([res.results[c]["out"] for c in range(NCORES)],
                            axis=0)
    out[tok_perm] = shards
    return out.reshape(1, S, TOUT)

